# revision 4
# baseline (speedup 1.0000x reference)
"""Classical self-attention on 8 trn2 NeuronCores — v3.

N=16384 tokens, d=64, fp32. Sequence-parallel over Q: core c handles rows
[c*2048, (c+1)*2048). Per-core differentiation via the xq input slice.

Algebra (host-folded):
  s_jq = k_j . q_q = x_j^T (G x_q + w) + const_q, G/w precomputed on host;
  the per-q const is absorbed by the softmax shift, so it is never computed.
  sum_j p_j v_j = [Wv|bv] @ (sum_j p_j [x_j;1])  -> V never materialized;
  the ones column of xh doubles as the denominator row.

Structure per core:
  setup: DMA x (8 chunks) + xq; PE-transpose x -> xT[65,16384] (ones row);
    fp8 shadows x8[32,2,N] (DoubleRow d-split: d = slot*32 + partition);
    g-proj gB[65,2048] = [G^T;w^T] @ xqT, fp8 shadow g8; pass A for group 0.
  pass A (row max): fp8 DR matmuls psA[128,1024] per (tile, 1024-key chunk);
    DVE tensor_tensor_reduce (max,max) -> mms[:,t,c]; mfin: reduce -> mt,
    PE transpose -> gB row 64 = -m̂.
  8 phases (groups of 256 q = 2 tiles): per unit (4 kv blocks):
    PV(u-2) x4, B x4 (fp32r, psB[:,u%2,:]), exp(u-1) on ACT, TTR(u-1),
    A-mm(u) for group h+1 last (spacing hides the single-buffered psA WAR).
  tail: psPV[65,256] -> OT; after phase 7: OTw = [Wv|bv]^T @ OT (+denom row),
    PE transpose, DVE reciprocal+scale, DMA out.
"""

import sys

sys.path.insert(0, "/opt/trn_rl_repo")

from contextlib import ExitStack

import numpy as np

import concourse.bass as bass
import concourse.mybir as mybir
import concourse.tile as tile
from concourse import bacc
from concourse.bass import ds, ts
from concourse.bass_utils import run_bass_kernel_spmd

N_CORES = 8
N = 16384
D = 64
QR = N // N_CORES          # 2048 q rows per core
N_QTILE = QR // 128        # 16 q tiles per core
N_PHASE = 8                # groups of 2 q-tiles (256 q cols)
N_KV_BLK = N // 128        # 128 kv blocks
N_UNIT = 32                # units per phase (4 blocks each)
N_ACH = 16                 # 1024-key A-chunks per q tile
F32 = mybir.dt.float32
F8 = mybir.dt.float8e4
R32 = mybir.dt.float32r
DR = mybir.MatmulPerfMode.DoubleRow
AX = mybir.AxisListType.X
MAX = mybir.AluOpType.max

_CACHED = {}


def build_kernel():
    nc = bacc.Bacc("TRN2", target_bir_lowering=False, debug=False,
                   num_devices=N_CORES)

    x_d = nc.dram_tensor("x", [N, D], F32, kind="ExternalInput")
    xq_d = nc.dram_tensor("xq", [QR, D], F32, kind="ExternalInput")
    gm_d = nc.dram_tensor("gmat", [D + 1, D], F32, kind="ExternalInput")
    wv_d = nc.dram_tensor("wvb", [D + 1, D], F32, kind="ExternalInput")
    id_d = nc.dram_tensor("ident", [128, 128], F32, kind="ExternalInput")
    on_d = nc.dram_tensor("ones", [2, N // 2], F32, kind="ExternalInput")
    y_d = nc.dram_tensor("y", [QR, D], F32, kind="ExternalOutput")

    def rb(ap):
        return ap.bitcast(R32)

    with tile.TileContext(nc) as tc, ExitStack() as ctx:
        sb = ctx.enter_context(tc.tile_pool(name="sb", bufs=1))
        expp = ctx.enter_context(tc.tile_pool(name="expp", bufs=3))
        smp = ctx.enter_context(tc.tile_pool(name="smp", bufs=4))
        scrp = ctx.enter_context(tc.tile_pool(name="scr", bufs=2))
        psB_p = ctx.enter_context(tc.tile_pool(name="psB", bufs=1, space="PSUM"))
        psA_p = ctx.enter_context(tc.tile_pool(name="psA", bufs=1, space="PSUM"))
        psPV_p = ctx.enter_context(tc.tile_pool(name="psPV", bufs=1, space="PSUM"))
        psM_p = ctx.enter_context(tc.tile_pool(name="psM", bufs=1, space="PSUM"))

        # ---- persistent SBUF ----
        xT = sb.tile([D + 1, N], F32)          # x^T, row 64 = ones
        x8 = sb.tile([32, 2, N], F8)           # DR d-split shadow of xT
        xV = sb.tile([128, N_KV_BLK, D + 1], F32)  # natural x, col 64 = ones
        xqn = sb.tile([128, N_QTILE, D], F32)  # natural xq (transpose source)
        xqT = sb.tile([D + 1, QR], F32)        # xq^T, row 64 = ones
        gB = sb.tile([D + 1, QR], F32)         # g, row 64 = -m̂
        g8 = sb.tile([32, 2, QR], F8)
        gmat = sb.tile([D + 1, D], F32)
        wvb = sb.tile([D + 1, D], F32)
        ident = sb.tile([128, 128], F32)
        mms = sb.tile([128, N_QTILE, N_ACH], F32)
        OT = sb.tile([D + 1, QR], F32)         # out1 = sum p [x;1]
        OTw = sb.tile([D + 1, QR], F32)        # [Wv|bv]-stage + denom row
        y_sb = sb.tile([128, N_QTILE, D], F32)

        # ---- PSUM ----
        psB = psB_p.tile([128, 2, 1024], F32)  # 4 banks, double-buffered
        psA = psA_p.tile([128, 1024], F32)     # 2 banks, single + spacing
        psPV = psPV_p.tile([128, 256], F32)    # 1 bank
        # psM: 1 bank scratch for g-proj / pneg / OTw / final transposes
        psm = psM_p.tile([128, 512], F32)

        # ---- DMAs (DMA_ENGINES serializes; order by first use) ----
        nc.gpsimd.dma_start(ident[:], id_d[:])
        nc.gpsimd.dma_start(gmat[:], gm_d[:])
        nc.gpsimd.dma_start(wvb[:], wv_d[:])
        nc.sync.dma_start(xqT[D : D + 1, 0 : QR], on_d[0:1, 0:QR])
        nc.sync.dma_start(
            xqn[:], xq_d[:].rearrange("(j p) d -> p j d", p=128))
        nc.sync.dma_start(xT[D : D + 1, 0 : N // 2], on_d[0:1, :])
        nc.sync.dma_start(xT[D : D + 1, N // 2 : N], on_d[1:2, :])
        for c in range(8):
            nc.sync.dma_start(
                xV[:, ds(c * 16, 16), 0:D],
                x_d[ds(c * 2048, 2048), :].rearrange("(j p) d -> p j d", p=128))
        nc.gpsimd.memset(xV[:, :, D : D + 1], 1.0)

        # ---- xq transposes (borrow psB halves) + g-proj + g8 ----
        for o in range(2):
            tgt = psB[:, o % 2, :]
            for j in range(8):
                nc.tensor.transpose(rb(tgt[0:D, ts(j, 128)]),
                                    rb(xqn[:, o * 8 + j, :]), rb(ident[:]))
            nc.vector.tensor_copy(xqT[0:D, ts(o, 1024)], tgt[0:D, :])
        for s in range(8):
            pm = psm[0:D, ds((s % 2) * 256, 256)]
            nc.tensor.matmul(pm, rb(gmat[:]), rb(xqT[:, ts(s, 256)]),
                             start=True, stop=True)
            nc.vector.tensor_copy(gB[0:D, ts(s, 256)], pm)
        nc.scalar.copy(g8[:, 0, :], gB[0:32, :])
        nc.scalar.copy(g8[:, 1, :], gB[32:64, :])

        # ---- pass-A plumbing ----
        mt_tiles = {}

        def emit_A(t, c):
            """fp8 DR matmul for (q tile t, 1024-key chunk c) -> psA."""
            nc.tensor.matmul(psA[:], g8[:, :, ts(t, 128)],
                             x8[:, :, ds(c * 1024, 1024)],
                             start=True, stop=True, perf_mode=DR)

        def emit_TTR(t, c):
            scr = scrp.tile([128, 512], F32, tag="scr")
            nc.vector.tensor_tensor_reduce(
                scr[:], psA[:, 0:512], psA[:, 512:1024], 1.0, -3.0e38,
                MAX, MAX, mms[:, t, c : c + 1])

        def emit_mfin(t):
            mt = smp.tile([128, 1], F32, tag="mt")
            nc.vector.reduce_max(mt[:], mms[:, t, :], axis=AX)
            pneg = psm[0:1, 0:128]
            nc.tensor.matmul(pneg, mt[:], ident[:], start=True, stop=True)
            nc.scalar.mul(gB[D : D + 1, ts(t, 128)], pneg, -1.0)

        # ---- setup: kv chunks -> xT, x8, pass A for group 0 ----
        a_prev = None
        for c in range(8):
            for oo in range(2):
                o = 2 * c + oo
                tgt = psB[:, o % 2, :]
                for j in range(8):
                    nc.tensor.transpose(rb(tgt[0:D, ts(j, 128)]),
                                        rb(xV[:, o * 8 + j, 0:D]),
                                        rb(ident[:]))
                if o % 2 == 0:
                    nc.scalar.copy(xT[0:D, ts(o, 1024)], tgt[0:D, :])
                else:
                    nc.vector.tensor_copy(xT[0:D, ts(o, 1024)], tgt[0:D, :])
                nc.scalar.copy(x8[:, 0, ts(o, 1024)], tgt[0:32, :])
                # Pool cannot read PSUM: slot-1 shadow reads xT after its copy
                nc.gpsimd.tensor_copy(x8[:, 1, ts(o, 1024)],
                                      xT[32:64, ts(o, 1024)])
                # interleave group-0 A work: chunk o for tiles 0 and 1
                for t in range(2):
                    if a_prev is not None:
                        emit_TTR(*a_prev)
                    emit_A(t, o)
                    a_prev = (t, o)
        emit_TTR(*a_prev)
        emit_mfin(0)
        emit_mfin(1)

        # ---- phases ----
        for h in range(N_PHASE):
            qsl = ds(h * 256, 256)
            atiles = (2 * (h + 1), 2 * (h + 1) + 1) if h < N_PHASE - 1 else None
            pv_q = []   # (blk, ex_ap) deferred two units
            ex_q = []   # unit indices awaiting exp, deferred one unit

            def emit_exp(u):
                ex = expp.tile([128, 1024], F32, tag="ex")
                nc.scalar.activation(ex[:], psB[:, u % 2, :],
                                     mybir.ActivationFunctionType.Exp)
                for j in range(4):
                    pv_q.append((4 * u + j, ex[:, ds(j * 256, 256)]))

            def emit_pv():
                while pv_q:
                    blk, ex_ap = pv_q.pop(0)
                    nc.tensor.matmul(psPV[0 : D + 1, :], rb(xV[:, blk, :]),
                                     rb(ex_ap), start=(blk == 0),
                                     stop=(blk == N_KV_BLK - 1),
                                     skip_group_check=True)

            for u in range(N_UNIT):
                emit_pv()
                for j in range(4):
                    blk = 4 * u + j
                    nc.tensor.matmul(psB[:, u % 2, ds(j * 256, 256)],
                                     rb(xT[:, ts(blk, 128)]), rb(gB[:, qsl]),
                                     start=True, stop=True)
                if ex_q:
                    emit_exp(ex_q.pop(0))
                if atiles is not None:
                    if u >= 1:
                        au = u - 1
                        emit_TTR(atiles[au // N_ACH], au % N_ACH)
                    emit_A(atiles[u // N_ACH], u % N_ACH)
                ex_q.append(u)
            # tail
            emit_exp(ex_q.pop(0))
            if atiles is not None:
                emit_TTR(atiles[1], N_ACH - 1)
            emit_pv()
            nc.vector.tensor_copy(OT[:, qsl], psPV[0 : D + 1, :])
            if atiles is not None:
                emit_mfin(atiles[0])
                emit_mfin(atiles[1])

        # ---- final: OTw = [Wv|bv]^T @ OT (+denom), transpose, normalize ----
        nc.scalar.copy(OTw[D : D + 1, :], OT[D : D + 1, :])
        for s in range(8):
            pm = psm[0:D, ds((s % 2) * 256, 256)]
            nc.tensor.matmul(pm, rb(wvb[:]), rb(OT[:, ts(s, 256)]),
                             start=True, stop=True)
            nc.vector.tensor_copy(OTw[0:D, ts(s, 256)], pm)
        for t in range(N_QTILE):
            pO = psB[:, t % 2, 0 : D + 1]
            nc.tensor.transpose(rb(pO), rb(OTw[:, ts(t, 128)]),
                                rb(ident[0 : D + 1, 0 : D + 1]))
            rz = smp.tile([128, 1], F32, tag="rz")
            nc.vector.reciprocal(rz[:], pO[:, D : D + 1])
            nc.vector.tensor_scalar_mul(y_sb[:, t, :], pO[:, 0:D], rz[:])
        nc.sync.dma_start(y_d.rearrange("(t p) d -> p t d", p=128), y_sb[:])

    nc.compile()
    return nc


def _prep_inputs(x, params, Wq, bq, Wk, bk, Wv, bv):
    f8s = np.float64
    x = np.ascontiguousarray(x, dtype=np.float32)
    params = np.asarray(params, f8s)
    rot = params[:, :D]
    ent = params[:, D : 2 * D]
    scale = 1.0 / np.sqrt(D)
    Wqp = (np.asarray(Wq, f8s) @ rot) * scale
    Wkp = np.asarray(Wk, f8s) @ ent
    bqs = np.asarray(bq, f8s) * scale
    G = Wkp.T @ Wqp
    w = Wkp.T @ bqs
    gmat = np.ascontiguousarray(
        np.vstack([G.T, w[None, :]]).astype(np.float32))
    wvb = np.ascontiguousarray(
        np.vstack([np.asarray(Wv, np.float32).T,
                   np.asarray(bv, np.float32)[None]]))
    ident = np.eye(128, dtype=np.float32)
    ones = np.ones([2, N // 2], dtype=np.float32)
    return x, gmat, wvb, ident, ones


def kernel(x, params, Wq, bq, Wk, bk, Wv, bv, _trace=False):
    x, gmat, wvb, ident, ones = _prep_inputs(x, params, Wq, bq, Wk, bk, Wv, bv)
    if "nc" not in _CACHED:
        _CACHED["nc"] = build_kernel()
    nc = _CACHED["nc"]
    in_maps = []
    for c in range(N_CORES):
        in_maps.append({
            "x": x,
            "xq": np.ascontiguousarray(x[c * QR : (c + 1) * QR]),
            "gmat": gmat, "wvb": wvb, "ident": ident, "ones": ones,
        })
    res = run_bass_kernel_spmd(nc, in_maps, core_ids=list(range(N_CORES)),
                               trace=_trace)
    out = np.concatenate([res.results[c]["y"] for c in range(N_CORES)], axis=0)
    global _CACHED_RES
    _CACHED_RES = res
    return out


# revision 7
# speedup vs baseline: 1.1694x; 1.1694x over previous
"""Classical self-attention on 8 trn2 NeuronCores — v3.

N=16384 tokens, d=64, fp32. Sequence-parallel over Q: core c handles rows
[c*2048, (c+1)*2048). Per-core differentiation via the xq input slice.

Algebra (host-folded):
  s_jq = k_j . q_q = x_j^T (G x_q + w) + const_q, G/w precomputed on host;
  the per-q const is absorbed by the softmax shift, so it is never computed.
  sum_j p_j v_j = [Wv|bv] @ (sum_j p_j [x_j;1])  -> V never materialized;
  the ones column of xh doubles as the denominator row.

Structure per core:
  setup: DMA x (8 chunks) + xq; PE-transpose x -> xT[65,16384] (ones row);
    fp8 shadows x8[32,2,N] (DoubleRow d-split: d = slot*32 + partition);
    g-proj gB[65,2048] = [G^T;w^T] @ xqT, fp8 shadow g8; pass A for group 0.
  pass A (row max): fp8 DR matmuls psA[128,1024] per (tile, 1024-key chunk);
    DVE tensor_tensor_reduce (max,max) -> mms[:,t,c]; mfin: reduce -> mt,
    PE transpose -> gB row 64 = -m̂.
  8 phases (groups of 256 q = 2 tiles): per unit (4 kv blocks):
    PV(u-2) x4, B x4 (fp32r, psB[:,u%2,:]), exp(u-1) on ACT, TTR(u-1),
    A-mm(u) for group h+1 last (spacing hides the single-buffered psA WAR).
  tail: psPV[65,256] -> OT; after phase 7: OTw = [Wv|bv]^T @ OT (+denom row),
    PE transpose, DVE reciprocal+scale, DMA out.
"""

import sys

sys.path.insert(0, "/opt/trn_rl_repo")

from contextlib import ExitStack

import numpy as np

import concourse.bass as bass
import concourse.mybir as mybir
import concourse.tile as tile
from concourse import bacc
from concourse.bass import ds, ts
from concourse.bass_utils import run_bass_kernel_spmd

N_CORES = 8
N = 16384
D = 64
QR = N // N_CORES          # 2048 q rows per core
N_QTILE = QR // 128        # 16 q tiles per core
N_PHASE = 8                # groups of 2 q-tiles (256 q cols)
N_KV_BLK = N // 128        # 128 kv blocks
N_UNIT = 32                # units per phase (4 blocks each)
N_ACH = 16                 # 1024-key A-chunks per q tile
F32 = mybir.dt.float32
F8 = mybir.dt.float8e4
R32 = mybir.dt.float32r
DR = mybir.MatmulPerfMode.DoubleRow
AX = mybir.AxisListType.X
MAX = mybir.AluOpType.max

_CACHED = {}


def build_kernel():
    nc = bacc.Bacc("TRN2", target_bir_lowering=False, debug=False,
                   num_devices=N_CORES)

    x_d = nc.dram_tensor("x", [N, D], F32, kind="ExternalInput")
    xq_d = nc.dram_tensor("xq", [QR, D], F32, kind="ExternalInput")
    gm_d = nc.dram_tensor("gmat", [D + 1, D], F32, kind="ExternalInput")
    wv_d = nc.dram_tensor("wvb", [D + 1, D], F32, kind="ExternalInput")
    id_d = nc.dram_tensor("ident", [128, 128], F32, kind="ExternalInput")
    on_d = nc.dram_tensor("ones", [2, N // 2], F32, kind="ExternalInput")
    y_d = nc.dram_tensor("y", [QR, D], F32, kind="ExternalOutput")

    def rb(ap):
        return ap.bitcast(R32)

    with tile.TileContext(nc) as tc, ExitStack() as ctx:
        sb = ctx.enter_context(tc.tile_pool(name="sb", bufs=1))
        expp = ctx.enter_context(tc.tile_pool(name="expp", bufs=3))
        smp = ctx.enter_context(tc.tile_pool(name="smp", bufs=4))
        scrp = ctx.enter_context(tc.tile_pool(name="scr", bufs=2))
        psB_p = ctx.enter_context(tc.tile_pool(name="psB", bufs=1, space="PSUM"))
        psA_p = ctx.enter_context(tc.tile_pool(name="psA", bufs=1, space="PSUM"))
        psPV_p = ctx.enter_context(tc.tile_pool(name="psPV", bufs=1, space="PSUM"))
        psM_p = ctx.enter_context(tc.tile_pool(name="psM", bufs=1, space="PSUM"))

        # ---- persistent SBUF ----
        xT = sb.tile([D + 1, N], F32)          # x^T, row 64 = ones
        x8 = sb.tile([32, 2, N], F8)           # DR d-split shadow of xT
        xV = sb.tile([128, N_KV_BLK, D + 1], F32)  # natural x, col 64 = ones
        xqn = sb.tile([128, N_QTILE, D], F32)  # natural xq (transpose source)
        xqT = sb.tile([D + 1, QR], F32)        # xq^T, row 64 = ones
        gB = sb.tile([D + 1, QR], F32)         # g, row 64 = -m̂
        g8 = sb.tile([32, 2, QR], F8)
        gmat = sb.tile([D + 1, D], F32)
        wvb = sb.tile([D + 1, D], F32)
        ident = sb.tile([128, 128], F32)
        mms = sb.tile([128, N_QTILE, N_ACH], F32)
        OT = sb.tile([D + 1, QR], F32)         # out1 = sum p [x;1]
        OTw = sb.tile([D + 1, QR], F32)        # [Wv|bv]-stage + denom row
        y_sb = sb.tile([128, N_QTILE, D], F32)

        # ---- PSUM ----
        psB = psB_p.tile([128, 2, 1024], F32)  # 4 banks, double-buffered
        psA = psA_p.tile([128, 1024], F32)     # 2 banks, single + spacing
        psPV = psPV_p.tile([128, 256], F32)    # 1 bank
        # psM: 1 bank scratch for g-proj / pneg / OTw / final transposes
        psm = psM_p.tile([128, 512], F32)

        # ---- DMAs (DMA_ENGINES serializes; order by first use) ----
        nc.gpsimd.dma_start(ident[:], id_d[:])
        nc.gpsimd.dma_start(gmat[:], gm_d[:])
        nc.gpsimd.dma_start(wvb[:], wv_d[:])
        nc.sync.dma_start(xqT[D : D + 1, 0 : QR], on_d[0:1, 0:QR])
        nc.sync.dma_start(
            xqn[:], xq_d[:].rearrange("(j p) d -> p j d", p=128))
        nc.sync.dma_start(xT[D : D + 1, 0 : N // 2], on_d[0:1, :])
        nc.sync.dma_start(xT[D : D + 1, N // 2 : N], on_d[1:2, :])
        for c in range(8):
            nc.sync.dma_start(
                xV[:, ds(c * 16, 16), 0:D],
                x_d[ds(c * 2048, 2048), :].rearrange("(j p) d -> p j d", p=128))
        nc.gpsimd.memset(xV[:, :, D : D + 1], 1.0)

        # ---- xq transposes (borrow psB halves) + g-proj + g8 ----
        for o in range(2):
            tgt = psB[:, o % 2, :]
            for j in range(8):
                nc.tensor.transpose(rb(tgt[0:D, ts(j, 128)]),
                                    rb(xqn[:, o * 8 + j, :]), rb(ident[:]))
            nc.vector.tensor_copy(xqT[0:D, ts(o, 1024)], tgt[0:D, :])
        for s in range(8):
            pm = psm[0:D, ds((s % 2) * 256, 256)]
            nc.tensor.matmul(pm, rb(gmat[:]), rb(xqT[:, ts(s, 256)]),
                             start=True, stop=True)
            nc.vector.tensor_copy(gB[0:D, ts(s, 256)], pm)
        nc.scalar.copy(g8[:, 0, :], gB[0:32, :])
        nc.scalar.copy(g8[:, 1, :], gB[32:64, :])

        # ---- pass-A plumbing ----
        mt_tiles = {}

        def emit_A(t, c):
            """fp8 DR matmul for (q tile t, 1024-key chunk c) -> psA."""
            nc.tensor.matmul(psA[:], g8[:, :, ts(t, 128)],
                             x8[:, :, ds(c * 1024, 1024)],
                             start=True, stop=True, perf_mode=DR)

        def emit_TTR(t, c):
            scr = scrp.tile([128, 512], F32, tag="scr")
            nc.vector.tensor_tensor_reduce(
                scr[:], psA[:, 0:512], psA[:, 512:1024], 1.0, -3.0e38,
                MAX, MAX, mms[:, t, c : c + 1])

        def emit_mfin(t):
            mt = smp.tile([128, 1], F32, tag="mt")
            nc.vector.reduce_max(mt[:], mms[:, t, :], axis=AX)
            pneg = psm[0:1, 0:128]
            nc.tensor.matmul(pneg, mt[:], ident[:], start=True, stop=True)
            nc.scalar.mul(gB[D : D + 1, ts(t, 128)], pneg, -1.0)

        # ---- setup: kv chunks -> xT, x8, pass A for group 0 ----
        a_prev = None
        for c in range(8):
            for oo in range(2):
                o = 2 * c + oo
                tgt = psB[:, o % 2, :]
                for j in range(8):
                    nc.tensor.transpose(rb(tgt[0:D, ts(j, 128)]),
                                        rb(xV[:, o * 8 + j, 0:D]),
                                        rb(ident[:]))
                nc.scalar.copy(xT[0:D, ts(o, 1024)], tgt[0:D, :])
                if o % 2 == 0:
                    nc.vector.tensor_copy(x8[:, 0, ts(o, 1024)], tgt[0:32, :])
                else:
                    nc.scalar.copy(x8[:, 0, ts(o, 1024)], tgt[0:32, :])
                # Pool cannot read PSUM: slot-1 shadow reads xT after its copy
                nc.gpsimd.tensor_copy(x8[:, 1, ts(o, 1024)],
                                      xT[32:64, ts(o, 1024)])
                # interleave group-0 A work: chunk o for tiles 0 and 1
                for t in range(2):
                    if a_prev is not None:
                        emit_TTR(*a_prev)
                    emit_A(t, o)
                    a_prev = (t, o)
        emit_TTR(*a_prev)
        emit_mfin(0)
        emit_mfin(1)

        # ---- phases ----
        for h in range(N_PHASE):
            qsl = ds(h * 256, 256)
            atiles = (2 * (h + 1), 2 * (h + 1) + 1) if h < N_PHASE - 1 else None
            pv_q = []   # (blk, ex_ap): emitted right after the NEXT unit's Bs

            def emit_exp(u):
                ex = expp.tile([128, 1024], F32, tag="ex")
                nc.scalar.activation(ex[:], psB[:, u % 2, :],
                                     mybir.ActivationFunctionType.Exp)
                for j in range(4):
                    pv_q.append((4 * u + j, ex[:, ds(j * 256, 256)]))

            def emit_pv():
                while pv_q:
                    blk, ex_ap = pv_q.pop(0)
                    nc.tensor.matmul(psPV[0 : D + 1, :], rb(xV[:, blk, :]),
                                     rb(ex_ap), start=(blk == 0),
                                     stop=(blk == N_KV_BLK - 1),
                                     skip_group_check=True)

            for u in range(N_UNIT):
                # Bs first: they feed this unit's exp with no other deps.
                for j in range(4):
                    blk = 4 * u + j
                    nc.tensor.matmul(psB[:, u % 2, ds(j * 256, 256)],
                                     rb(xT[:, ts(blk, 128)]), rb(gB[:, qsl]),
                                     start=True, stop=True)
                # PVs of unit u-1: their exp finished during this unit's Bs.
                emit_pv()
                emit_exp(u)
                if atiles is not None:
                    if u >= 1:
                        au = u - 1
                        emit_TTR(atiles[au // N_ACH], au % N_ACH)
                    emit_A(atiles[u // N_ACH], u % N_ACH)
            # tail
            if atiles is not None:
                emit_TTR(atiles[1], N_ACH - 1)
            emit_pv()
            nc.vector.tensor_copy(OT[:, qsl], psPV[0 : D + 1, :])
            if atiles is not None:
                emit_mfin(atiles[0])
                emit_mfin(atiles[1])

        # ---- final: OTw = [Wv|bv]^T @ OT (+denom), transpose, normalize ----
        nc.scalar.copy(OTw[D : D + 1, :], OT[D : D + 1, :])
        for s in range(8):
            pm = psm[0:D, ds((s % 2) * 256, 256)]
            nc.tensor.matmul(pm, rb(wvb[:]), rb(OT[:, ts(s, 256)]),
                             start=True, stop=True)
            nc.vector.tensor_copy(OTw[0:D, ts(s, 256)], pm)
        for t in range(N_QTILE):
            pO = psB[:, t % 2, 0 : D + 1]
            nc.tensor.transpose(rb(pO), rb(OTw[:, ts(t, 128)]),
                                rb(ident[0 : D + 1, 0 : D + 1]))
            rz = smp.tile([128, 1], F32, tag="rz")
            nc.vector.reciprocal(rz[:], pO[:, D : D + 1])
            nc.vector.tensor_scalar_mul(y_sb[:, t, :], pO[:, 0:D], rz[:])
        nc.sync.dma_start(y_d.rearrange("(t p) d -> p t d", p=128), y_sb[:])

    nc.compile()
    return nc


def _prep_inputs(x, params, Wq, bq, Wk, bk, Wv, bv):
    f8s = np.float64
    x = np.ascontiguousarray(x, dtype=np.float32)
    params = np.asarray(params, f8s)
    rot = params[:, :D]
    ent = params[:, D : 2 * D]
    scale = 1.0 / np.sqrt(D)
    Wqp = (np.asarray(Wq, f8s) @ rot) * scale
    Wkp = np.asarray(Wk, f8s) @ ent
    bqs = np.asarray(bq, f8s) * scale
    G = Wkp.T @ Wqp
    w = Wkp.T @ bqs
    gmat = np.ascontiguousarray(
        np.vstack([G.T, w[None, :]]).astype(np.float32))
    wvb = np.ascontiguousarray(
        np.vstack([np.asarray(Wv, np.float32).T,
                   np.asarray(bv, np.float32)[None]]))
    ident = np.eye(128, dtype=np.float32)
    ones = np.ones([2, N // 2], dtype=np.float32)
    return x, gmat, wvb, ident, ones


def kernel(x, params, Wq, bq, Wk, bk, Wv, bv, _trace=False):
    x, gmat, wvb, ident, ones = _prep_inputs(x, params, Wq, bq, Wk, bk, Wv, bv)
    if "nc" not in _CACHED:
        _CACHED["nc"] = build_kernel()
    nc = _CACHED["nc"]
    in_maps = []
    for c in range(N_CORES):
        in_maps.append({
            "x": x,
            "xq": np.ascontiguousarray(x[c * QR : (c + 1) * QR]),
            "gmat": gmat, "wvb": wvb, "ident": ident, "ones": ones,
        })
    res = run_bass_kernel_spmd(nc, in_maps, core_ids=list(range(N_CORES)),
                               trace=_trace)
    out = np.concatenate([res.results[c]["y"] for c in range(N_CORES)], axis=0)
    global _CACHED_RES
    _CACHED_RES = res
    return out
